# revision 19
# baseline (speedup 1.0000x reference)
"""Trainium2 Bass kernel: 8-head causal MHA with RoPE, B=2 T=2048 E=1024 H=8 D=512.

Sharding: 8 cores = 2 (batch) x 4 (head-pair) tensor-parallel groups.
Each core computes q/k/v projections for its 2 heads, causal attention,
and a row-parallel partial of the output projection; the host sums the
4 partials per batch (unshard) and transposes back to [B, T, E].

All matmul operands are float16 (f32 PSUM accumulation), which runs the PE
at full rate, halves HBM traffic vs f32, and enables the DVE 4x fast mode
for the rope arithmetic (all-SBUF 2-byte operands). RoPE's even/odd pairing
is pre-baked into the wq/wk rows host-side (scores are invariant under a
shared permutation of the head dim) and the 1/sqrt(D) scale is folded into
wq. Scores are computed transposed (keys on partitions) so softmax probs
feed attn@V and the wo matmul with no on-chip transposes. Causal structure
skips fully-masked 128x256 score blocks; the two diagonal blocks per
query-block get a multiplicative 0/1 f16 mask after exp. Input DMAs are
issued in first-use order so the PE starts ~5us into the kernel instead of
waiting for the full x transfer.
"""

import os
import sys

for _p in ("/opt/trn_rl_repo", "/root/.axon_site/_ro/trn_rl_repo"):
    if os.path.isdir(_p) and _p not in sys.path:
        sys.path.insert(0, _p)

import numpy as np

import concourse.bacc as bacc
import concourse.mybir as mybir
import concourse.tile as tile
from concourse.bass_utils import run_bass_kernel_spmd

B, T, E, H, D = 2, 2048, 1024, 8, 512
P = 128
NE = E // P          # 8 e-chunks (contraction)
NTB = T // 512       # 4 t-blocks of 512
NTT = T // P         # 16 t-tiles of 128
NDC = D // P         # 4 d-chunks per head
HPC = 2              # heads per core
G = 4                # head groups (cores per batch)
IBW = 256            # attention query-block width
NIB = T // IBW       # 8 query blocks

F32 = mybir.dt.float32
F16 = mybir.dt.float16

BLOCKS_STD = [(tb * 512, 512) for tb in range(NTB)]

# repeat the whole body R times inside the NEFF (device-time measurement via
# wall-clock amplification; always 1 for real use)
KREPS = int(os.environ.get("KREPS", "1"))


def _build_nc():
    nc = bacc.Bacc("TRN2", target_bir_lowering=False, debug=False, num_devices=8)

    xT = nc.declare_dram_parameter("xT", [E, T], F16, isOutput=False)
    wqT = nc.declare_dram_parameter("wqT", [E, HPC * D], F16, isOutput=False)
    wkT = nc.declare_dram_parameter("wkT", [E, HPC * D], F16, isOutput=False)
    wvT = nc.declare_dram_parameter("wvT", [E, HPC * D], F16, isOutput=False)
    woT = nc.declare_dram_parameter("woT", [HPC * D, E], F16, isOutput=False)
    cosT = nc.declare_dram_parameter("cosT", [D // 2, T], F16, isOutput=False)
    sinT = nc.declare_dram_parameter("sinT", [D // 2, T], F16, isOutput=False)
    masks = nc.declare_dram_parameter("masks", [2 * P, IBW], F16, isOutput=False)
    outT = nc.declare_dram_parameter("outT", [E, T], F16, isOutput=True)

    xr = xT.rearrange("(c p) t -> p c t", p=P)
    cr = cosT.rearrange("(d p) t -> p d t", p=P)
    sr = sinT.rearrange("(d p) t -> p d t", p=P)

    with tile.TileContext(nc) as tc:
        with (
            tc.tile_pool(name="glob", bufs=1) as gp,
            tc.tile_pool(name="right", bufs=1, side="right") as rp,
            tc.tile_pool(name="psum", bufs=1, space="PSUM") as pp,
        ):
            def sb(name, w=512, dtype=F16, tag="rs", bufs=8):
                return gp.tile([P, w], dtype, tag=tag, bufs=bufs, name=name)

            def proj_qk(h, ws, dst, cos_t, sin_t, xt, tname, blocks=BLOCKS_STD):
                """q/k projection with fused rope into dst [P, NDC, T] (f16)."""
                for dp in range(2):
                    for tb, (c0, w) in enumerate(blocks):
                        cols = slice(c0, c0 + w)
                        psA = pp.tile([P, 512], F32, tag="acc", bufs=4,
                                      name=f"psA_{tname}{h}{dp}{tb}")
                        psB = pp.tile([P, 512], F32, tag="acc", bufs=4,
                                      name=f"psB_{tname}{h}{dp}{tb}")
                        for c in range(NE):
                            nc.tensor.matmul(psA[:, 0:w],
                                             ws[:, c, dp * 256 : dp * 256 + 128],
                                             xt[:, c, cols],
                                             start=(c == 0), stop=(c == NE - 1))
                        for c in range(NE):
                            nc.tensor.matmul(psB[:, 0:w],
                                             ws[:, c, dp * 256 + 128 : dp * 256 + 256],
                                             xt[:, c, cols],
                                             start=(c == 0), stop=(c == NE - 1))
                        sA = sb(f"sA{h}{dp}{tb}{tname}")
                        sB = sb(f"sB{h}{dp}{tb}{tname}")
                        nc.scalar.activation(sA[:, 0:w], psA[:, 0:w],
                                             mybir.ActivationFunctionType.Copy)
                        nc.scalar.activation(sB[:, 0:w], psB[:, 0:w],
                                             mybir.ActivationFunctionType.Copy)
                        ct = cos_t[:, dp, cols]
                        st = sin_t[:, dp, cols]
                        t_ac = sb(f"tac{h}{dp}{tb}{tname}", tag="rt")
                        t_bs = sb(f"tbs{h}{dp}{tb}{tname}", tag="rt")
                        nc.vector.tensor_mul(t_ac[:, 0:w], sA[:, 0:w], ct)
                        nc.vector.tensor_mul(t_bs[:, 0:w], sB[:, 0:w], st)
                        nc.vector.tensor_sub(dst[:, dp, cols], t_ac[:, 0:w], t_bs[:, 0:w])
                        t_as = sb(f"tas{h}{dp}{tb}{tname}", tag="rt")
                        t_bc = sb(f"tbc{h}{dp}{tb}{tname}", tag="rt")
                        nc.vector.tensor_mul(t_as[:, 0:w], sA[:, 0:w], st)
                        nc.vector.tensor_mul(t_bc[:, 0:w], sB[:, 0:w], ct)
                        nc.vector.tensor_add(dst[:, dp + 2, cols], t_as[:, 0:w], t_bc[:, 0:w])

            def proj_v(h, ws, vv, xt):
                for tt in range(NTT):
                    psV = pp.tile([P, 512], F32, tag="acc", bufs=4, name=f"psV{h}{tt}")
                    for c in range(NE):
                        nc.tensor.matmul(psV[:], xt[:, c, tt * P : (tt + 1) * P],
                                         ws[:, c, :],
                                         start=(c == 0), stop=(c == NE - 1))
                    nc.scalar.activation(vv[:, tt, :], psV[:],
                                         mybir.ActivationFunctionType.Copy)

            def attention(h, qr, kr, vv, oT, mask_t, ones_t):
                """causal attention; writes normalized f16 oT[:, 4h..4h+4, :]."""
                for ib in range(NIB):
                    icols = slice(ib * IBW, (ib + 1) * IBW)
                    po = [pp.tile([P, 512], F32, tag="acc", bufs=4,
                                  name=f"po{h}{ib}{half}") for half in range(2)]
                    pd = pp.tile([1, IBW], F32, tag="pd", bufs=1, name=f"pd{h}{ib}")
                    jt_max = 2 * ib + 1
                    for jt in range(jt_max + 1):
                        ps = pp.tile([P, IBW], F32, tag="ps", bufs=3,
                                     name=f"ps{h}{ib}{jt}")
                        for dc in range(NDC):
                            nc.tensor.matmul(ps[:],
                                             kr[:, dc, jt * P : (jt + 1) * P],
                                             qr[:, dc, icols],
                                             start=(dc == 0), stop=(dc == NDC - 1))
                        e_t = sb(f"et{h}{ib}{jt}", IBW, tag="et")
                        nc.scalar.activation(e_t[:], ps[:],
                                             mybir.ActivationFunctionType.Exp)
                        q = jt - 2 * ib
                        if q >= 0:
                            nc.vector.tensor_mul(e_t[:], e_t[:], mask_t[:, q, :])
                        for dc in range(NDC):
                            nc.tensor.matmul(po[dc // 2][:, (dc % 2) * IBW : (dc % 2 + 1) * IBW],
                                             vv[:, jt, dc * P : (dc + 1) * P],
                                             e_t[:],
                                             start=(jt == 0 and dc % 2 == 0),
                                             stop=(jt == jt_max and dc % 2 == 1))
                        nc.tensor.matmul(pd[:], ones_t[:], e_t[:],
                                         start=(jt == 0), stop=(jt == jt_max))
                    # softmax denominator: reciprocal on [1,IBW], then broadcast
                    rb1 = sb(f"rb1{h}{ib}", IBW, F32, tag="rb1", bufs=2)
                    nc.vector.reciprocal_approx_fast(rb1[0:1, :], pd[:])
                    rbb = sb(f"rbb{h}{ib}", IBW, F32, tag="rbb", bufs=2)
                    nc.gpsimd.partition_broadcast(rbb[:], rb1[0:1, :])
                    for dc in range(NDC):
                        src = po[dc // 2][:, (dc % 2) * IBW : (dc % 2 + 1) * IBW]
                        nc.vector.tensor_mul(oT[:, 4 * h + dc, icols], src, rbb[:])

            for rep in range(KREPS):
                qr = rp.tile([P, NDC, T], F16, tag="qr", name="qr")
                kr = rp.tile([P, NDC, T], F16, tag="kr", name="kr")
                vv = rp.tile([P, NTT, D], F16, tag="vv", name="vv")
                oT = rp.tile([P, 2 * NDC, T], F16, tag="oT", name="oT")
                mask_t = rp.tile([P, 2, IBW], F16, tag="masks", name="mask_t")
                ones_t = rp.tile([P, 1], F16, tag="ones", name="ones_t")
                nc.vector.memset(ones_t[:], 1.0)

                def wsld(name, wdram, h, pieces=1):
                    """weight load, optionally in 128-col pieces matching the
                    A0/B0/A1/B1 use order (for the startup-critical load)."""
                    ws = gp.tile([P, NE, 512], F16, tag="ws", bufs=3, name=name)
                    wr = wdram.rearrange("(c p) d -> p c d", p=P)
                    pw = 512 // pieces
                    for i in range(pieces):
                        nc.sync.dma_start(
                            ws[:, :, i * pw : (i + 1) * pw],
                            wr[:, :, h * D + i * pw : h * D + (i + 1) * pw])
                    return ws

                with tc.tile_pool(name="left", bufs=1) as lp:
                    xt = lp.tile([P, NE, T], F16, tag="xt")
                    cos_t = lp.tile([P, 2, T], F16, tag="cos")
                    sin_t = lp.tile([P, 2, T], F16, tag="sin")
                    # issue order = first-use order: first half x t-block and
                    # the first 128-col q-weight slice (all the first matmul
                    # group needs), then the rest in consumption order.
                    nc.sync.dma_start(xt[:, :, 0:512], xr[:, :, 0:512])
                    ws_q0 = wsld("ws_q0", wqT, 0, pieces=4)
                    nc.sync.dma_start(cos_t[:, :, 0:512], cr[:, :, 0:512])
                    nc.sync.dma_start(sin_t[:, :, 0:512], sr[:, :, 0:512])
                    for tb in range(1, NTB):
                        cols = slice(tb * 512, (tb + 1) * 512)
                        nc.sync.dma_start(xt[:, :, cols], xr[:, :, cols])
                        nc.sync.dma_start(cos_t[:, :, cols], cr[:, :, cols])
                        nc.sync.dma_start(sin_t[:, :, cols], sr[:, :, cols])
                    nc.sync.dma_start(mask_t[:], masks.rearrange("(q p) c -> p q c", p=P))
                    ws_k0 = wsld("ws_k0", wkT, 0)
                    ws_v0 = wsld("ws_v0", wvT, 0)

                    # head 0
                    proj_qk(0, ws_q0, qr, cos_t, sin_t, xt, "q")
                    proj_qk(0, ws_k0, kr, cos_t, sin_t, xt, "k")
                    proj_v(0, ws_v0, vv, xt)
                    attention(0, qr, kr, vv, oT, mask_t, ones_t)

                    # head 1 (reuses qr/kr/vv slots; WAR deps order vs attn 0)
                    qr1 = rp.tile([P, NDC, T], F16, tag="qr", name="qr1")
                    kr1 = rp.tile([P, NDC, T], F16, tag="kr", name="kr1")
                    vv1 = rp.tile([P, NTT, D], F16, tag="vv", name="vv1")
                    ws_q1 = wsld("ws_q1", wqT, 1)
                    ws_k1 = wsld("ws_k1", wkT, 1)
                    ws_v1 = wsld("ws_v1", wvT, 1)
                    proj_qk(1, ws_q1, qr1, cos_t, sin_t, xt, "q")
                    proj_qk(1, ws_k1, kr1, cos_t, sin_t, xt, "k")
                    proj_v(1, ws_v1, vv1, xt)

                # left pool released: x/trig space becomes the wo-phase pool
                with tc.tile_pool(name="left2", bufs=1) as lp2:
                    wo_t = lp2.tile([P, NE, E], F16, tag="wo_t")
                    wor = woT.rearrange("(c p) e -> p c e", p=P)
                    nc.sync.dma_start(wo_t[:, 0:4, :], wor[:, 0:4, :])
                    nc.sync.dma_start(wo_t[:, 4:8, :], wor[:, 4:8, :])

                    attention(1, qr1, kr1, vv1, oT, mask_t, ones_t)

                    # output projection: contraction over both heads' 1024 hd
                    for tb in range(NTB):
                        cols = slice(tb * 512, (tb + 1) * 512)
                        for et in range(NE):
                            pw = pp.tile([P, 512], F32, tag="acc", bufs=4,
                                         name=f"pw{tb}{et}")
                            for hc in range(NE):
                                nc.tensor.matmul(
                                    pw[:], wo_t[:, hc, et * P : (et + 1) * P],
                                    oT[:, hc, cols],
                                    start=(hc == 0), stop=(hc == NE - 1))
                            ow = sb(f"ow{tb}{et}", tag="ow", bufs=6)
                            nc.scalar.activation(ow[:], pw[:],
                                                 mybir.ActivationFunctionType.Copy)
                            nc.sync.dma_start(outT[et * P : (et + 1) * P, cols], ow[:])

    nc.compile()
    return nc


_NC = None


def _get_nc():
    global _NC
    if _NC is None:
        _NC = _build_nc()
    return _NC


def _prep_inputs(x, wq, wk, wv, wo):
    """Host-side shard prep. Returns in_maps list of 8 dicts (core = b*4+g)."""
    x = np.asarray(x, dtype=np.float32)
    wq = np.asarray(wq, dtype=np.float32)
    wk = np.asarray(wk, dtype=np.float32)
    wv = np.asarray(wv, dtype=np.float32)
    wo = np.asarray(wo, dtype=np.float32)

    # rope permutation of head-dim rows: per head, new order =
    # [pair-block 0 x1 | pair-block 0 x2 | pair-block 1 x1 | pair-block 1 x2]
    perm = np.empty(D, dtype=np.int64)
    for dp in range(2):
        base = dp * 256
        pairs = dp * 128 + np.arange(128)
        perm[base : base + 128] = 2 * pairs
        perm[base + 128 : base + 256] = 2 * pairs + 1
    full_perm = np.concatenate([h * D + perm for h in range(H)])

    scale = 1.0 / np.sqrt(np.float32(D))
    wq_p = (wq[full_perm] * scale).astype(np.float16)
    wk_p = wk[full_perm].astype(np.float16)
    wv16 = wv.astype(np.float16)
    wo16 = wo.astype(np.float16)
    x16 = x.astype(np.float16)

    # rope tables [D/2, T] f16 (pair-index major)
    inv_freq = 1.0 / (10000.0 ** (np.arange(0, D, 2, dtype=np.float64) / D))
    ang = inv_freq[:, None] * np.arange(T, dtype=np.float64)[None, :]
    cosT = np.cos(ang).astype(np.float16)
    sinT = np.sin(ang).astype(np.float16)

    # multiplicative 0/1 causal masks for the 2 diagonal 128x256 sub-blocks
    rj = np.arange(P)[:, None]
    c = np.arange(IBW)[None, :]
    masks = np.empty((2 * P, IBW), dtype=np.float16)
    for q in range(2):
        masks[q * P : (q + 1) * P] = (c >= 128 * q + rj).astype(np.float16)

    in_maps = []
    for core in range(8):
        b, g = divmod(core, G)
        rows = slice(g * HPC * D, (g + 1) * HPC * D)
        in_maps.append({
            "xT": np.ascontiguousarray(x16[b].T),
            "wqT": np.ascontiguousarray(wq_p[rows].T),
            "wkT": np.ascontiguousarray(wk_p[rows].T),
            "wvT": np.ascontiguousarray(wv16[rows].T),
            "woT": np.ascontiguousarray(wo16[:, rows].T),
            "cosT": cosT,
            "sinT": sinT,
            "masks": masks,
        })
    return in_maps


def _assemble(results):
    """Sum the 4 TP partials per batch and transpose back to [B, T, E]."""
    out = np.empty((B, T, E), dtype=np.float32)
    for b in range(B):
        acc = results[b * G]["outT"].astype(np.float32)
        for g in range(1, G):
            acc = acc + results[b * G + g]["outT"].astype(np.float32)
        out[b] = acc.T
    return out


def kernel(x, wq, wk, wv, wo):
    nc = _get_nc()
    in_maps = _prep_inputs(x, wq, wk, wv, wo)
    res = run_bass_kernel_spmd(nc, in_maps, list(range(8)))
    return _assemble(res.results)
